# revision 24
# baseline (speedup 1.0000x reference)
"""GRU decoder kernel for Trainium2 (8 NeuronCores, data-parallel over batch).

Math (PyTorch GRU, gate order r,z,n), per batch element:
    gx_t = x_t * w_ih + b_ih              (input dim == 1 -> rank-1)
    gh_t = h_{t-1} @ w_hh.T + b_hh
    r = sigmoid(gx_r + gh_r); z = sigmoid(gx_z + gh_z)
    n = tanh(gx_n + b_ih_n + r * (gh_n + b_hh_n))
    h_t = (1-z)*n + z*h_{t-1}
    out = h_T @ fc_w.T + fc_b

Device layout (per core, B_c = 1024 batch):
  - partition-stacked: batch 0-511 ("u") on SBUF partitions 0-63,
    batch 512-1023 ("v") on partitions 64-127.  Elementwise tiles are
    [128, HG] (gate dim on partitions within each half, batch free).
  - Two PE quadrant chains: u on tile_position (0,0), v on (64,64);
    matmuls emitted u/v-interleaved so the quadrants run concurrently.
  - PSUM banks are double-buffered on step parity: the x one-hot
    matmuls for step t+1 issue during step t (they don't depend on h),
    so only the three W.h matmul pairs sit on the recurrent chain.
  - Z gate negated on host so sigmoid(-zpre, bias=-bz) = 1-z directly.
  - n's x-contribution precomputed on host: xb2[p,t,b] = wn[p]*x[t,b]
    + bni[p] (f16), streamed by chunked contiguous DMA; consumed by a
    plain fp16 tensor_add (2x DVE mode).
  - h' = h + (1-z)*(n - h): sub (optionally on GpSimd), mul, add.
  - fp16 SBUF tensors, fp32 PSUM accumulation.
"""

import os
import sys

sys.path.insert(0, "/opt/trn_rl_repo")

import numpy as np
from contextlib import ExitStack

HIDDEN = 64
OUT = 256
B = 8192
T = int(os.environ.get("GRU_T", 1024))
NCORES = 8
BC = B // NCORES          # 1024 batch per core
HB = BC // 2              # 512 batch per partition-half
UNROLL = 64               # steps per loop body (one-hot q index is static)
NGROUP = int(os.environ.get("GRU_NGROUP", 2))  # phase-shifted batch groups
NBLK = T // UNROLL        # number of 64-step blocks
HG = HB // NGROUP         # free-dim width per pipelined batch group
CH = 16                   # xb2 chunk: steps per DMA
NCHUNK = UNROLL // CH     # chunks per block (4)
USE_GP = os.environ.get("GRU_GP", "1") == "1"  # C = NN - H on GpSimd
PREISSUE = os.environ.get("GRU_PRE", "1") == "1"  # one-hots one step early
V3COMPAT = os.environ.get("GRU_V3", "0") == "1"  # v3 exact mm order/flags

_CACHE = {}


def _build():
    import concourse.bass as bass
    import concourse.tile as tile
    from concourse import bacc, mybir

    f16 = mybir.dt.float16
    f32 = mybir.dt.float32
    AF = mybir.ActivationFunctionType
    OP = mybir.AluOpType

    nc = bacc.Bacc("TRN2", target_bir_lowering=False, debug=False,
                   num_devices=NCORES)

    d_x = nc.dram_tensor("xt", [128, NBLK, HB], f16,
                         kind="ExternalInput").ap()
    d_xb = nc.dram_tensor("xb2", [NBLK + 1, 128, UNROLL, HB], f16,
                          kind="ExternalInput").ap()
    d_wr = nc.dram_tensor("wr", [128, 64], f16, kind="ExternalInput").ap()
    d_wzn = nc.dram_tensor("wzn", [128, 64], f16, kind="ExternalInput").ap()
    d_wn = nc.dram_tensor("wn", [128, 64], f16, kind="ExternalInput").ap()
    d_ohr = nc.dram_tensor("ohr", [128, UNROLL, 64], f16, kind="ExternalInput").ap()
    d_ohzn = nc.dram_tensor("ohzn", [128, UNROLL, 64], f16, kind="ExternalInput").ap()
    d_br = nc.dram_tensor("br", [128, 1], f32, kind="ExternalInput").ap()
    d_bzn = nc.dram_tensor("bzn", [128, 1], f32, kind="ExternalInput").ap()
    d_bnh = nc.dram_tensor("bnh", [128, 1], f32, kind="ExternalInput").ap()
    d_fcw = nc.dram_tensor("fcw", [128, OUT], f16, kind="ExternalInput").ap()
    d_fcb = nc.dram_tensor("fcb", [128, 2], f32, kind="ExternalInput").ap()
    d_out = nc.dram_tensor("out", [OUT, BC], f32, kind="ExternalOutput").ap()

    with tile.TileContext(nc) as tc, ExitStack() as ctx:
        singles = ctx.enter_context(tc.tile_pool(name="singles", bufs=1))
        xbpool = ctx.enter_context(tc.tile_pool(name="xbpool", bufs=1))
        work = ctx.enter_context(tc.tile_pool(name="work", bufs=4))
        psum = ctx.enter_context(tc.tile_pool(name="psum", bufs=1, space="PSUM"))

        X = singles.tile([128, NBLK, HB], f16)
        WR = singles.tile([128, 64], f16)
        WZN = singles.tile([128, 64], f16)
        WN = singles.tile([128, 64], f16)
        OHR = singles.tile([128, UNROLL, 64], f16)
        OHZN = singles.tile([128, UNROLL, 64], f16)
        BR = singles.tile([128, 1], f32)
        BZN = singles.tile([128, 1], f32)
        BNH = singles.tile([128, 1], f32)
        FCW = singles.tile([128, OUT], f16)
        FCB = singles.tile([128, 2], f32)
        H = singles.tile([128, HB], f16)

        for dst, src in ((X, d_x), (WR, d_wr), (WZN, d_wzn), (WN, d_wn),
                         (OHR, d_ohr), (OHZN, d_ohzn),
                         (BR, d_br), (BZN, d_bzn), (BNH, d_bnh),
                         (FCW, d_fcw), (FCB, d_fcb)):
            nc.gpsimd.dma_start(dst[:], src[:])
        nc.vector.memset(H[:], 0.0)

        xb_tiles = [xbpool.tile([128, CH, HB], f16, tag=f"xb{k}",
                                name=f"xb{k}")
                    for k in range(NCHUNK)]

        def xb_load(tag_idx, blk_ap, c_lo):
            t = xb_tiles[tag_idx]
            wblk, lo = c_lo // UNROLL, c_lo % UNROLL
            nc.gpsimd.dma_start(t[:], blk_ap[wblk, :, lo:lo + CH, :])

        def rz_banks():
            return [psum.tile([128, 2 * HG], f32, tag=f"bankRZ{g}",
                              name=f"bankRZ{g}")
                    for g in range(NGROUP)]

        def step_mms(q, xsb, g, bankRZ, bankN):
            """v3-verified pattern: per quadrant, each PSUM region gets
            its own [start=True ... stop=True] group每 step, W.h first
            (R group first so sigma-R's deps complete earliest)."""
            gc = slice(g * HG, (g + 1) * HG)
            mms = []
            for lo in (0, 64):
                sl = slice(lo, lo + 64)
                tp = (lo, lo)
                hs = H[sl, gc]
                xs = xsb[sl, :, gc]
                mms.append((
                    (bankRZ[sl, 0:HG], WR[sl, :], hs, True, False, tp),
                    (bankRZ[sl, 0:HG], OHR[sl, q, :], xs, False, True, tp),
                    (bankN[sl, :], WN[sl, :], hs, True, True, tp),
                    (bankRZ[sl, HG:2 * HG], WZN[sl, :], hs, True, False, tp),
                    (bankRZ[sl, HG:2 * HG], OHZN[sl, q, :], xs, False, True, tp),
                ))
            for mu, mv in zip(*mms):
                for out, lhsT, rhs, st, sp, tp in (mu, mv):
                    nc.tensor.matmul(out, lhsT, rhs, start=st, stop=sp,
                                     tile_position=tp)

        def body(blk, blk2):
            xsb = X[:, blk, :]
            for q in range(UNROLL):
                c, ql = q // CH, q % CH
                if ql == 0:
                    nxt = c + NCHUNK - 1
                    xb_load((nxt % NCHUNK), blk2, nxt * CH)
                bankRZ = rz_banks()
                bankN = [psum.tile([128, HG], f32, tag=f"bankN{g}",
                                   name=f"bankN{g}")
                         for g in range(NGROUP)]
                SR = [work.tile([128, HG], f16, tag=f"SR{g}", name=f"SR{g}")
                      for g in range(NGROUP)]
                SZC = [work.tile([128, HG], f16, tag=f"SZC{g}", name=f"SZC{g}")
                       for g in range(NGROUP)]
                T1 = [work.tile([128, HG], f16, tag=f"T1{g}", name=f"T1{g}")
                      for g in range(NGROUP)]
                T2 = [work.tile([128, HG], f16, tag=f"T2{g}", name=f"T2{g}")
                      for g in range(NGROUP)]
                NN = [work.tile([128, HG], f16, tag=f"NN{g}", name=f"NN{g}")
                      for g in range(NGROUP)]
                C = [work.tile([128, HG], f16, tag=f"C{g}", name=f"C{g}")
                     for g in range(NGROUP)]
                D = [work.tile([128, HG], f16, tag=f"D{g}", name=f"D{g}")
                     for g in range(NGROUP)]
                for g in range(NGROUP):
                    step_mms(q, xsb, g, bankRZ[g], bankN[g])
                for g in range(NGROUP):
                    nc.scalar.activation(SR[g][:], bankRZ[g][:, 0:HG],
                                         AF.Sigmoid, bias=BR[:])
                    nc.scalar.activation(SZC[g][:], bankRZ[g][:, HG:2 * HG],
                                         AF.Sigmoid, bias=BZN[:])
                for g in range(NGROUP):
                    gc = slice(g * HG, (g + 1) * HG)
                    nc.vector.scalar_tensor_tensor(
                        T1[g][:], bankN[g][:], BNH[:], SR[g][:],
                        op0=OP.add, op1=OP.mult)
                    nc.vector.tensor_add(T2[g][:], T1[g][:],
                                         xb_tiles[c][:, ql, gc])
                for g in range(NGROUP):
                    nc.scalar.activation(NN[g][:], T2[g][:], AF.Tanh)
                for g in range(NGROUP):
                    gc = slice(g * HG, (g + 1) * HG)
                    if USE_GP:
                        nc.gpsimd.tensor_sub(C[g][:], NN[g][:], H[:, gc])
                    else:
                        nc.vector.tensor_sub(C[g][:], NN[g][:], H[:, gc])
                    nc.vector.tensor_mul(D[g][:], SZC[g][:], C[g][:])
                    nc.vector.tensor_add(H[:, gc], H[:, gc], D[g][:])

        # prologue: xb chunks 0..2 of block 0
        win0 = d_xb[0:2, :, :, :]
        for k in range(NCHUNK - 1):
            xb_load(k, win0, k * CH)

        if NBLK == 1:
            body(0, win0)
        else:
            with tc.For_i(0, NBLK, 1,
                          hint_engines=(mybir.EngineType.PE,)) as i:
                body(bass.ds(i, 1), d_xb[bass.ds(i, 2), :, :, :])

        # Final FC: out[o, b] = sum_k fc_w[o, k] h[b, k] + fc_b[o]
        for oh in range(2):
            osl = slice(oh * 128, (oh + 1) * 128)
            fc_u = psum.tile([128, HB], f32, tag="bankRZ0", name="fc_u")
            fc_v = psum.tile([128, HB], f32, tag="bankRZ1", name="fc_v")
            nc.tensor.matmul(fc_u[:], FCW[0:64, osl], H[0:64, :],
                             start=True, stop=True, tile_position=(0, 0))
            nc.tensor.matmul(fc_v[:], FCW[64:128, osl], H[64:128, :],
                             start=True, stop=True, tile_position=(64, 0))
            Ou = work.tile([128, HB], f32, tag="Ou", name="Ou")
            Ov = work.tile([128, HB], f32, tag="Ov", name="Ov")
            nc.scalar.activation(Ou[:], fc_u[:], AF.Identity,
                                 bias=FCB[:, oh:oh + 1])
            nc.scalar.activation(Ov[:], fc_v[:], AF.Identity,
                                 bias=FCB[:, oh:oh + 1])
            nc.gpsimd.dma_start(d_out[osl, 0:HB], Ou[:])
            nc.gpsimd.dma_start(d_out[osl, HB:BC], Ov[:])

    nc.compile()
    return nc


def _host_inputs(x, w_ih, w_hh, b_ih, b_hh, fc_w, fc_b):
    """Build the per-core in_maps (numpy, laid out exactly as SBUF tiles)."""
    f16 = np.float16
    f32 = np.float32
    x = np.asarray(x, f32)
    w_ih = np.asarray(w_ih, f32)
    w_hh = np.asarray(w_hh, f32)
    b_ih = np.asarray(b_ih, f32)
    b_hh = np.asarray(b_hh, f32)
    fc_w = np.asarray(fc_w, f32)
    fc_b = np.asarray(fc_b, f32)

    eye = np.eye(UNROLL, dtype=f32)

    def oh(seg, sign=1.0):
        w = sign * w_ih[seg, 0]
        o = np.einsum("pq,m->pqm", eye, w)            # [64, UNROLL, 64]
        return np.concatenate([o, o], 0).astype(f16)  # [128, UNROLL, 64]

    def wstack(seg, sign=1.0):
        t = sign * w_hh[seg, :].T                     # [64(k), 64(m)]
        return np.vstack([t, t]).astype(f16)

    def btile(v):
        return np.tile(v.reshape(-1, 1), (2, 1)).astype(f32)  # [128, 1]

    br = b_ih[0:64] + b_hh[0:64]
    bz = b_ih[64:128] + b_hh[64:128]
    wn = w_ih[128:192, 0]                             # [64]
    bni = b_ih[128:192]

    shared = {
        "wr": wstack(slice(0, 64)),
        "wzn": wstack(slice(64, 128), -1.0),
        "wn": wstack(slice(128, 192)),
        "ohr": oh(slice(0, 64)),
        "ohzn": oh(slice(64, 128), -1.0),
        "br": btile(br),
        "bzn": btile(-bz),
        "bnh": btile(b_hh[128:192]),
        "fcw": np.vstack([fc_w.T, fc_w.T]).astype(f16),  # [128, 256]
        "fcb": np.stack([fc_b[0:128], fc_b[128:256]], 1).astype(f32),
    }
    wn2 = np.concatenate([wn, wn])                    # [128]
    bni2 = np.concatenate([bni, bni])                 # [128]

    in_maps = []
    for c in range(NCORES):
        xs = x[c * BC:(c + 1) * BC, :T, 0]            # [BC b, T t]
        xT = np.ascontiguousarray(xs.T)               # [T, BC]
        xr = xT.reshape(NBLK, UNROLL, BC)             # [blk, p, b]
        lo = xr[:, :, 0:HB].transpose(1, 0, 2)        # [64, blk, HB]
        hi = xr[:, :, HB:BC].transpose(1, 0, 2)
        Xh = np.ascontiguousarray(
            np.concatenate([lo, hi], 0)).astype(f16)  # [128, blk, HB]
        # xb2[blk, p, t, b] = wn2[p] * x[t, b-half(p)] + bni2[p]
        xuv = np.stack([xr[:, :, 0:HB], xr[:, :, HB:BC]], 1)  # [blk, 2, t, b]
        xb2 = np.empty((NBLK + 1, 128, UNROLL, HB), f16)
        xb2[NBLK] = 0.0
        half = np.repeat(np.arange(2), 64)            # [128]
        xb2[:NBLK] = (wn2[None, :, None, None] * xuv[:, half, :, :]
                      + bni2[None, :, None, None]).astype(f16)
        m = dict(shared)
        m["xt"] = Xh
        m["xb2"] = xb2
        in_maps.append(m)
    return in_maps


def _run(in_maps, trace=False):
    from concourse import bass_utils
    if "nc" not in _CACHE:
        _CACHE["nc"] = _build()
    nc = _CACHE["nc"]
    res = bass_utils.run_bass_kernel_spmd(
        nc, in_maps, core_ids=list(range(NCORES)), trace=trace)
    return res


def kernel(**inputs):
    in_maps = _host_inputs(**inputs)
    res = _run(in_maps, trace=False)
    out = np.empty([B, OUT], np.float32)
    for c in range(NCORES):
        out[c * BC:(c + 1) * BC, :] = res.results[c]["out"].T
    return out


# revision 26
# speedup vs baseline: 1.1327x; 1.1327x over previous
"""GRU decoder kernel for Trainium2 (8 NeuronCores, data-parallel over batch).

Math (PyTorch GRU, gate order r,z,n), per batch element:
    gx_t = x_t * w_ih + b_ih              (input dim == 1 -> rank-1)
    gh_t = h_{t-1} @ w_hh.T + b_hh
    r = sigmoid(gx_r + gh_r); z = sigmoid(gx_z + gh_z)
    n = tanh(gx_n + b_ih_n + r * (gh_n + b_hh_n))
    h_t = (1-z)*n + z*h_{t-1}
    out = h_T @ fc_w.T + fc_b

Device layout (per core, B_c = 1024 batch):
  - partition-stacked: batch 0-511 ("u") on SBUF partitions 0-63,
    batch 512-1023 ("v") on partitions 64-127.  Elementwise tiles are
    [128, HG] (gate dim on partitions within each half, batch free).
  - Two PE quadrant chains: u on tile_position (0,0), v on (64,64);
    matmuls emitted u/v-interleaved so the quadrants run concurrently.
  - PSUM banks are double-buffered on step parity: the x one-hot
    matmuls for step t+1 issue during step t (they don't depend on h),
    so only the three W.h matmul pairs sit on the recurrent chain.
  - Z gate negated on host so sigmoid(-zpre, bias=-bz) = 1-z directly.
  - n's x-contribution precomputed on host: xb2[p,t,b] = wn[p]*x[t,b]
    + bni[p] (f16), streamed by chunked contiguous DMA; consumed by a
    plain fp16 tensor_add (2x DVE mode).
  - h' = h + (1-z)*(n - h): sub (optionally on GpSimd), mul, add.
  - fp16 SBUF tensors, fp32 PSUM accumulation.
"""

import os
import sys

sys.path.insert(0, "/opt/trn_rl_repo")

import numpy as np
from contextlib import ExitStack

HIDDEN = 64
OUT = 256
B = 8192
T = int(os.environ.get("GRU_T", 1024))
NCORES = 8
BC = B // NCORES          # 1024 batch per core
HB = BC // 2              # 512 batch per partition-half
UNROLL = 64               # steps per loop body (one-hot q index is static)
NGROUP = int(os.environ.get("GRU_NGROUP", 2))  # phase-shifted batch groups
NBLK = T // UNROLL        # number of 64-step blocks
HG = HB // NGROUP         # free-dim width per pipelined batch group
CH = 16                   # xb2 chunk: steps per DMA
NCHUNK = UNROLL // CH     # chunks per block (4)
USE_GP = os.environ.get("GRU_GP", "0") == "1"  # C = NN - H on GpSimd
PREISSUE = os.environ.get("GRU_PRE", "1") == "1"  # one-hots one step early
V3COMPAT = os.environ.get("GRU_V3", "0") == "1"  # v3 exact mm order/flags

_CACHE = {}


def _build():
    import concourse.bass as bass
    import concourse.tile as tile
    from concourse import bacc, mybir

    f16 = mybir.dt.float16
    f32 = mybir.dt.float32
    AF = mybir.ActivationFunctionType
    OP = mybir.AluOpType

    nc = bacc.Bacc("TRN2", target_bir_lowering=False, debug=False,
                   num_devices=NCORES)

    d_x = nc.dram_tensor("xt", [128, NBLK, HB], f16,
                         kind="ExternalInput").ap()
    d_xb = nc.dram_tensor("xb2", [NBLK + 1, 128, UNROLL, HB], f16,
                          kind="ExternalInput").ap()
    d_wr = nc.dram_tensor("wr", [128, 64], f16, kind="ExternalInput").ap()
    d_wzn = nc.dram_tensor("wzn", [128, 64], f16, kind="ExternalInput").ap()
    d_wn = nc.dram_tensor("wn", [128, 64], f16, kind="ExternalInput").ap()
    d_ohr = nc.dram_tensor("ohr", [128, UNROLL, 64], f16, kind="ExternalInput").ap()
    d_ohzn = nc.dram_tensor("ohzn", [128, UNROLL, 64], f16, kind="ExternalInput").ap()
    d_br = nc.dram_tensor("br", [128, 1], f32, kind="ExternalInput").ap()
    d_bzn = nc.dram_tensor("bzn", [128, 1], f32, kind="ExternalInput").ap()
    d_bnh = nc.dram_tensor("bnh", [128, 1], f32, kind="ExternalInput").ap()
    d_fcw = nc.dram_tensor("fcw", [128, OUT], f16, kind="ExternalInput").ap()
    d_fcb = nc.dram_tensor("fcb", [128, 2], f32, kind="ExternalInput").ap()
    d_out = nc.dram_tensor("out", [OUT, BC], f32, kind="ExternalOutput").ap()

    with tile.TileContext(nc) as tc, ExitStack() as ctx:
        singles = ctx.enter_context(tc.tile_pool(name="singles", bufs=1))
        xbpool = ctx.enter_context(tc.tile_pool(name="xbpool", bufs=1))
        work = ctx.enter_context(tc.tile_pool(name="work", bufs=4))
        psum = ctx.enter_context(tc.tile_pool(name="psum", bufs=1, space="PSUM"))

        X = singles.tile([128, NBLK, HB], f16)
        WR = singles.tile([128, 64], f16)
        WZN = singles.tile([128, 64], f16)
        WN = singles.tile([128, 64], f16)
        OHR = singles.tile([128, UNROLL, 64], f16)
        OHZN = singles.tile([128, UNROLL, 64], f16)
        BR = singles.tile([128, 1], f32)
        BZN = singles.tile([128, 1], f32)
        BNH = singles.tile([128, 1], f32)
        FCW = singles.tile([128, OUT], f16)
        FCB = singles.tile([128, 2], f32)
        H = singles.tile([128, HB], f16)

        for dst, src in ((X, d_x), (WR, d_wr), (WZN, d_wzn), (WN, d_wn),
                         (OHR, d_ohr), (OHZN, d_ohzn),
                         (BR, d_br), (BZN, d_bzn), (BNH, d_bnh),
                         (FCW, d_fcw), (FCB, d_fcb)):
            nc.gpsimd.dma_start(dst[:], src[:])
        nc.vector.memset(H[:], 0.0)

        xb_tiles = [xbpool.tile([128, CH, HB], f16, tag=f"xb{k}",
                                name=f"xb{k}")
                    for k in range(NCHUNK)]

        def xb_load(tag_idx, blk_ap, c_lo):
            t = xb_tiles[tag_idx]
            wblk, lo = c_lo // UNROLL, c_lo % UNROLL
            nc.gpsimd.dma_start(t[:], blk_ap[wblk, :, lo:lo + CH, :])

        def rz_banks(nm):
            return [psum.tile([128, HG], f32, tag=f"bank{nm}{g}",
                              name=f"bank{nm}{g}")
                    for g in range(NGROUP)]

        def step_mms(q, xsb, g, bankR, bankZ, bankN):
            """v3-verified pattern: per quadrant, each PSUM region gets
            its own [start=True ... stop=True] group每 step, W.h first
            (R group first so sigma-R's deps complete earliest)."""
            gc = slice(g * HG, (g + 1) * HG)
            mms = []
            for lo in (0, 64):
                sl = slice(lo, lo + 64)
                tp = (lo, lo)
                hs = H[sl, gc]
                xs = xsb[sl, :, gc]
                mms.append((
                    (bankR[sl, :], WR[sl, :], hs, True, False, tp),
                    (bankR[sl, :], OHR[sl, q, :], xs, False, True, tp),
                    (bankN[sl, :], WN[sl, :], hs, True, True, tp),
                    (bankZ[sl, :], WZN[sl, :], hs, True, False, tp),
                    (bankZ[sl, :], OHZN[sl, q, :], xs, False, True, tp),
                ))
            for mu, mv in zip(*mms):
                for out, lhsT, rhs, st, sp, tp in (mu, mv):
                    nc.tensor.matmul(out, lhsT, rhs, start=st, stop=sp,
                                     tile_position=tp)

        def body(blk, blk2):
            xsb = X[:, blk, :]
            for q in range(UNROLL):
                c, ql = q // CH, q % CH
                if ql == 0:
                    nxt = c + NCHUNK - 1
                    xb_load((nxt % NCHUNK), blk2, nxt * CH)
                bankR = rz_banks("R")
                bankZ = rz_banks("Z")
                bankN = rz_banks("N")
                SR = [work.tile([128, HG], f16, tag=f"SR{g}", name=f"SR{g}")
                      for g in range(NGROUP)]
                SZC = [work.tile([128, HG], f16, tag=f"SZC{g}", name=f"SZC{g}")
                       for g in range(NGROUP)]
                T1 = [work.tile([128, HG], f16, tag=f"T1{g}", name=f"T1{g}")
                      for g in range(NGROUP)]
                T2 = [work.tile([128, HG], f16, tag=f"T2{g}", name=f"T2{g}")
                      for g in range(NGROUP)]
                NN = [work.tile([128, HG], f16, tag=f"NN{g}", name=f"NN{g}")
                      for g in range(NGROUP)]
                C = [work.tile([128, HG], f16, tag=f"C{g}", name=f"C{g}")
                     for g in range(NGROUP)]
                D = [work.tile([128, HG], f16, tag=f"D{g}", name=f"D{g}")
                     for g in range(NGROUP)]
                for g in range(NGROUP):
                    step_mms(q, xsb, g, bankR[g], bankZ[g], bankN[g])
                for g in range(NGROUP):
                    nc.scalar.activation(SR[g][:], bankR[g][:],
                                         AF.Sigmoid, bias=BR[:])
                for g in range(NGROUP):
                    nc.scalar.activation(SZC[g][:], bankZ[g][:],
                                         AF.Sigmoid, bias=BZN[:])
                for g in range(NGROUP):
                    gc = slice(g * HG, (g + 1) * HG)
                    nc.vector.scalar_tensor_tensor(
                        T1[g][:], bankN[g][:], BNH[:], SR[g][:],
                        op0=OP.add, op1=OP.mult)
                    nc.vector.tensor_add(T2[g][:], T1[g][:],
                                         xb_tiles[c][:, ql, gc])
                for g in range(NGROUP):
                    nc.scalar.activation(NN[g][:], T2[g][:], AF.Tanh)
                for g in range(NGROUP):
                    gc = slice(g * HG, (g + 1) * HG)
                    if USE_GP:
                        nc.gpsimd.tensor_sub(C[g][:], NN[g][:], H[:, gc])
                    else:
                        nc.vector.tensor_sub(C[g][:], NN[g][:], H[:, gc])
                    nc.vector.tensor_mul(D[g][:], SZC[g][:], C[g][:])
                    nc.vector.tensor_add(H[:, gc], H[:, gc], D[g][:])

        # prologue: xb chunks 0..2 of block 0
        win0 = d_xb[0:2, :, :, :]
        for k in range(NCHUNK - 1):
            xb_load(k, win0, k * CH)

        if NBLK == 1:
            body(0, win0)
        else:
            with tc.For_i(0, NBLK, 1,
                          hint_engines=(mybir.EngineType.PE,)) as i:
                body(bass.ds(i, 1), d_xb[bass.ds(i, 2), :, :, :])

        # Final FC: out[o, b] = sum_k fc_w[o, k] h[b, k] + fc_b[o]
        for oh in range(2):
            osl = slice(oh * 128, (oh + 1) * 128)
            fc_u = psum.tile([128, HB], f32, tag="fcu", name="fc_u")
            fc_v = psum.tile([128, HB], f32, tag="fcv", name="fc_v")
            nc.tensor.matmul(fc_u[:], FCW[0:64, osl], H[0:64, :],
                             start=True, stop=True, tile_position=(0, 0))
            nc.tensor.matmul(fc_v[:], FCW[64:128, osl], H[64:128, :],
                             start=True, stop=True, tile_position=(64, 0))
            Ou = work.tile([128, HB], f32, tag="Ou", name="Ou")
            Ov = work.tile([128, HB], f32, tag="Ov", name="Ov")
            nc.scalar.activation(Ou[:], fc_u[:], AF.Identity,
                                 bias=FCB[:, oh:oh + 1])
            nc.scalar.activation(Ov[:], fc_v[:], AF.Identity,
                                 bias=FCB[:, oh:oh + 1])
            nc.gpsimd.dma_start(d_out[osl, 0:HB], Ou[:])
            nc.gpsimd.dma_start(d_out[osl, HB:BC], Ov[:])

    nc.compile()
    return nc


def _host_inputs(x, w_ih, w_hh, b_ih, b_hh, fc_w, fc_b):
    """Build the per-core in_maps (numpy, laid out exactly as SBUF tiles)."""
    f16 = np.float16
    f32 = np.float32
    x = np.asarray(x, f32)
    w_ih = np.asarray(w_ih, f32)
    w_hh = np.asarray(w_hh, f32)
    b_ih = np.asarray(b_ih, f32)
    b_hh = np.asarray(b_hh, f32)
    fc_w = np.asarray(fc_w, f32)
    fc_b = np.asarray(fc_b, f32)

    eye = np.eye(UNROLL, dtype=f32)

    def oh(seg, sign=1.0):
        w = sign * w_ih[seg, 0]
        o = np.einsum("pq,m->pqm", eye, w)            # [64, UNROLL, 64]
        return np.concatenate([o, o], 0).astype(f16)  # [128, UNROLL, 64]

    def wstack(seg, sign=1.0):
        t = sign * w_hh[seg, :].T                     # [64(k), 64(m)]
        return np.vstack([t, t]).astype(f16)

    def btile(v):
        return np.tile(v.reshape(-1, 1), (2, 1)).astype(f32)  # [128, 1]

    br = b_ih[0:64] + b_hh[0:64]
    bz = b_ih[64:128] + b_hh[64:128]
    wn = w_ih[128:192, 0]                             # [64]
    bni = b_ih[128:192]

    shared = {
        "wr": wstack(slice(0, 64)),
        "wzn": wstack(slice(64, 128), -1.0),
        "wn": wstack(slice(128, 192)),
        "ohr": oh(slice(0, 64)),
        "ohzn": oh(slice(64, 128), -1.0),
        "br": btile(br),
        "bzn": btile(-bz),
        "bnh": btile(b_hh[128:192]),
        "fcw": np.vstack([fc_w.T, fc_w.T]).astype(f16),  # [128, 256]
        "fcb": np.stack([fc_b[0:128], fc_b[128:256]], 1).astype(f32),
    }
    wn2 = np.concatenate([wn, wn])                    # [128]
    bni2 = np.concatenate([bni, bni])                 # [128]

    in_maps = []
    for c in range(NCORES):
        xs = x[c * BC:(c + 1) * BC, :T, 0]            # [BC b, T t]
        xT = np.ascontiguousarray(xs.T)               # [T, BC]
        xr = xT.reshape(NBLK, UNROLL, BC)             # [blk, p, b]
        lo = xr[:, :, 0:HB].transpose(1, 0, 2)        # [64, blk, HB]
        hi = xr[:, :, HB:BC].transpose(1, 0, 2)
        Xh = np.ascontiguousarray(
            np.concatenate([lo, hi], 0)).astype(f16)  # [128, blk, HB]
        # xb2[blk, p, t, b] = wn2[p] * x[t, b-half(p)] + bni2[p]
        xuv = np.stack([xr[:, :, 0:HB], xr[:, :, HB:BC]], 1)  # [blk, 2, t, b]
        xb2 = np.empty((NBLK + 1, 128, UNROLL, HB), f16)
        xb2[NBLK] = 0.0
        half = np.repeat(np.arange(2), 64)            # [128]
        xb2[:NBLK] = (wn2[None, :, None, None] * xuv[:, half, :, :]
                      + bni2[None, :, None, None]).astype(f16)
        m = dict(shared)
        m["xt"] = Xh
        m["xb2"] = xb2
        in_maps.append(m)
    return in_maps


def _run(in_maps, trace=False):
    from concourse import bass_utils
    if "nc" not in _CACHE:
        _CACHE["nc"] = _build()
    nc = _CACHE["nc"]
    res = bass_utils.run_bass_kernel_spmd(
        nc, in_maps, core_ids=list(range(NCORES)), trace=trace)
    return res


def kernel(**inputs):
    in_maps = _host_inputs(**inputs)
    res = _run(in_maps, trace=False)
    out = np.empty([B, OUT], np.float32)
    for c in range(NCORES):
        out[c * BC:(c + 1) * BC, :] = res.results[c]["out"].T
    return out
